# revision 8
# baseline (speedup 1.0000x reference)
"""Causal self-attention with RoPE, sharded over 8 TRN2 NeuronCores.

Sharding: data-parallel over B (4 ways) x tensor-parallel over heads
(2 ways, 6 heads each). Each core computes qkv projection, RoPE,
causal attention and a partial output projection for its (batch,
head-half); the host sums the two head-half partials per batch.

Structure: one interleaved PE program. The softmax exp (the
scalar-engine invariant, ~13.4M elements/core) is taken off the
critical path by interleaving qkv / attention-V / output-projection
matmuls ("shadow work") between score matmuls. Slabs of 512 queries
are processed head-pair-major: pair hp's score stream is shadowed by
the previous pair's attention-V matmuls, so only the last pair's AV
remains as tail. DMA issues cost ~600ns of sequencer each, so
transfers are batched via 3D access patterns (x/w/rope tables/consts
land in one DMA per group). Attention weights are stored as fp8e4
(exp applies a -3.5 bias; the softmax ratio is bias-invariant). The
j=0 key block keeps bf16 weights: rows q<128 can have all-small
scores whose fp8 weights would flush to zero and NaN the denominator;
for q>=128 the row max is > -3.4 a.s. so fp8 is safe.

Matmuls run in bf16 (f32 PSUM accumulate); attention matmuls are
uniform (K=128, M=128): kT is zero-padded per head to a 128-partition
stationary, V carries a ones column for the softmax denominator. RoPE
pairs are host-permuted into contiguous even/odd halves per head.
"""

import math

import numpy as np
import ml_dtypes

import concourse.bass as bass
import concourse.tile as tile
import concourse.mybir as mybir
from concourse import bacc
from concourse.bass_utils import run_bass_kernel_spmd

B, T, C, H, D = 4, 2048, 768, 12, 64
HL = H // 2          # heads per core
TB = T // 128        # 16 t-blocks
CB = C // 128        # 6 contraction blocks
NCORES = 8
NSLAB = 4            # slabs of 512 queries

F32 = mybir.dt.float32
BF16 = mybir.dt.bfloat16
FP8 = mybir.dt.float8e4
AF = mybir.ActivationFunctionType

EXP_BIAS = -3.5      # exp(s*0.125 + bias); cancels in softmax ratio
PP_BUFS = 21         # fp8 pP tiles in flight per head-pair tag

_CACHED_NC = None


def build_nc():
    nc = bacc.Bacc("TRN2", target_bir_lowering=False)

    xT = nc.declare_dram_parameter("xT", [C, T], BF16, isOutput=False)
    wqkvT = nc.declare_dram_parameter("wqkvT", [C, 3 * HL * D], BF16, isOutput=False)
    wpT = nc.declare_dram_parameter("wpT", [HL * D, C], BF16, isOutput=False)
    sinr = nc.declare_dram_parameter("sinr", [T, HL * D], BF16, isOutput=False)
    cosr = nc.declare_dram_parameter("cosr", [T, HL * D], BF16, isOutput=False)
    # [tri | negi | ident] packed: one DMA
    consts = nc.declare_dram_parameter("consts", [128, 384], BF16, isOutput=False)
    out = nc.declare_dram_parameter("out", [T, C], BF16, isOutput=True)

    xTv = xT.rearrange("(cb p) t -> p cb t", cb=CB)
    wv = wqkvT.rearrange("(cb p) f -> p cb f", cb=CB)
    wpv = wpT.rearrange("(fb p) f -> p fb f", fb=3)
    sinv = sinr.rearrange("(tb p) f -> p tb f", tb=TB)
    cosv = cosr.rearrange("(tb p) f -> p tb f", tb=TB)

    with tile.TileContext(nc) as tc:
        with (
            tc.tile_pool(name="persist", bufs=1) as persist,
            tc.tile_pool(name="pxin", bufs=2) as pxin,
            tc.tile_pool(name="ptab", bufs=2) as ptab,
            tc.tile_pool(name="ropew", bufs=2) as ropew,
            tc.tile_pool(name="pPf", bufs=PP_BUFS) as pPf,
            tc.tile_pool(name="pPb", bufs=3) as pPb,
            tc.tile_pool(name="small", bufs=2) as small,
            tc.tile_pool(name="pout", bufs=2) as pout,
            tc.tile_pool(name="psA", bufs=2, space="PSUM") as psA,
            tc.tile_pool(name="psB", bufs=2, space="PSUM") as psB,
            tc.tile_pool(name="psC", bufs=1, space="PSUM") as psC,
            tc.tile_pool(name="psO", bufs=1, space="PSUM") as psO,
        ):
            # ---- persistent SBUF tensors ----
            qT_all = persist.tile([128, 3 * T], BF16, tag="qT", name="qT_all")
            kTp_all = persist.tile([128, HL * T], BF16, tag="kT", name="kTp_all")
            v_sb = [persist.tile([128, HL, 128], BF16, tag=f"v{i}", name=f"v{i}")
                    for i in range(TB)]
            w_sb = persist.tile([128, CB, 3 * HL * D], BF16, tag="w", name="w_sb")
            wp_sb = persist.tile([128, 3, C], BF16, tag="wp", name="wp_sb")
            yT128 = [persist.tile([128, T], BF16, tag=f"y128_{i}", name=f"y128_{i}")
                     for i in range(3)]
            cst = persist.tile([128, 3, 128], BF16, tag="cst", name="cst")
            bias_sb = persist.tile([128, 1], F32, tag="bias")
            tri_sb = cst[:, 0, :]
            negi_sb = cst[:, 1, :]
            id_sb = cst[:, 2, :]

            kTp_v = kTp_all.rearrange("p (c two t) -> p c two t", c=3, two=2)
            qv = qT_all.rearrange("p (c t) -> p c t", c=3)

            # ---- preload (few, large DMAs; issue cost is ~600ns each) ----
            xtiles = {}
            sctabs = {}

            def stage_x(g):
                if g >= NSLAB:
                    return
                t = pxin.tile([128, CB, 512], BF16, tag="x", name="x")
                nc.sync.dma_start(out=t, in_=xTv[:, :, g * 512:(g + 1) * 512])
                xtiles[g] = t

            def stage_sincos(g):
                if g >= NSLAB:
                    return
                s = ptab.tile([128, 4, HL * D], BF16, tag="sin", name="sin")
                c = ptab.tile([128, 4, HL * D], BF16, tag="cos", name="cos")
                nc.gpsimd.dma_start(out=s, in_=sinv[:, 4 * g:4 * g + 4, :])
                nc.gpsimd.dma_start(out=c, in_=cosv[:, 4 * g:4 * g + 4, :])
                sctabs[g] = (s, c)

            nc.gpsimd.dma_start(out=cst, in_=consts.rearrange("p (i f) -> p i f", i=3))
            nc.gpsimd.memset(bias_sb, EXP_BIAS)
            stage_sincos(0)
            stage_x(0)
            nc.sync.dma_start(out=w_sb[:, :, 0:384], in_=wv[:, :, 0:384])
            stage_x(1)
            nc.sync.dma_start(out=w_sb[:, :, 384:], in_=wv[:, :, 384:])
            nc.gpsimd.dma_start(out=wp_sb, in_=wpv)
            stage_sincos(1)

            # ---- emit helpers ----
            pt_tiles = {}

            def qkv_chunk(tb, chunk):
                g = tb // 4
                toff = (tb % 4) * 128
                mm = psB.tile([128, 384], F32, tag="mm", name="mm")
                for cb in range(CB):
                    nc.tensor.matmul(
                        mm,
                        lhsT=xtiles[g][:, cb, toff:toff + 128],
                        rhs=w_sb[:, cb, chunk * 384:(chunk + 1) * 384],
                        start=(cb == 0),
                        stop=(cb == CB - 1),
                    )
                if chunk == 2:
                    nc.scalar.copy(
                        v_sb[tb][:, :, 0:D],
                        mm.rearrange("p (h d) -> p h d", h=HL),
                    )
                    return
                if chunk == 0:
                    pt_tiles[tb] = psC.tile([128, 768], BF16, tag="pt", name="pt")
                sin_sb = sctabs[g][0][:, tb % 4, :]
                cos_sb = sctabs[g][1][:, tb % 4, :]
                pt = pt_tiles[tb]
                ro = ropew.tile([128, HL * D], BF16, tag=f"ro{chunk}",
                                name=f"ro{chunk}")
                t1 = ropew.tile([128, HL * D], BF16, tag="t1", name="t1")
                t2 = ropew.tile([128, HL * D], BF16, tag="t2", name="t2")
                nc.vector.tensor_mul(t1, mm, cos_sb)
                nc.vector.tensor_mul(t2, mm, sin_sb)
                rv = ro.rearrange("p (h half i) -> p h half i", h=HL, half=2)
                t1v = t1.rearrange("p (h half i) -> p h half i", h=HL, half=2)
                t2v = t2.rearrange("p (h half i) -> p h half i", h=HL, half=2)
                nc.vector.tensor_sub(rv[:, :, 0:1, :], t1v[:, :, 0:1, :],
                                     t2v[:, :, 1:2, :])
                nc.vector.tensor_add(rv[:, :, 1:2, :], t2v[:, :, 0:1, :],
                                     t1v[:, :, 1:2, :])
                off = chunk * 384
                for cb2 in range(3):
                    nc.tensor.transpose(
                        pt[:, off + cb2 * 128:off + (cb2 + 1) * 128],
                        ro[:, cb2 * 128:(cb2 + 1) * 128], id_sb
                    )
                tsl = bass.ts(tb, 128)
                ptv = pt[:, off:off + 384].rearrange("p (c t) -> p c t", c=3)
                if chunk == 0:
                    nc.scalar.copy(qv[:, :, tsl], ptv)
                else:
                    nc.scalar.copy(kTp_v[0:64, :, 0:1, tsl], ptv[0:64, :, :])
                    nc.scalar.copy(kTp_v[64:128, :, 1:2, tsl], ptv[64:128, :, :])
                    del pt_tiles[tb]

            pP_live = {}

            def scores(g, j, hp):
                d = j - 4 * g
                diag = d >= 0
                Nj = 512 - max(0, d) * 128
                qa = g * 512 + max(0, d) * 128
                sc = psA.tile([128, 1024], F32, tag="sc", name="sc")
                for e in (0, 1):
                    h = 2 * hp + e
                    o = e * 512
                    nc.tensor.matmul(
                        sc[:, o:o + Nj],
                        lhsT=kTp_all[:, h * T + j * 128:h * T + (j + 1) * 128],
                        rhs=qT_all[:, hp * T + qa:hp * T + qa + Nj],
                        start=True, stop=not diag,
                    )
                    if diag:
                        nc.tensor.matmul(
                            sc[:, o:o + 128],
                            lhsT=negi_sb, rhs=tri_sb,
                            start=False, stop=True,
                            skip_group_check=True,
                        )
                if g == 0 and j == 0:
                    pP = pPb.tile([128, 1024], BF16, tag="pPb", name="pPb")
                else:
                    pP = pPf.tile([128, 1024], FP8, tag=f"pP{hp}", name=f"pP{hp}")
                if Nj == 512:
                    nc.scalar.activation(pP[:, 0:1024], sc[:, 0:1024], AF.Exp,
                                         bias=bias_sb, scale=0.125)
                else:
                    for e in (0, 1):
                        o = e * 512
                        nc.scalar.activation(pP[:, o:o + Nj], sc[:, o:o + Nj],
                                             AF.Exp, bias=bias_sb, scale=0.125)
                pP_live[(g, j, hp)] = pP

            po_live = {}

            def av_unit(g, h, js):
                hp, e = h // 2, h % 2
                for j in js:
                    d = j - 4 * g
                    Nj = 512 - max(0, d) * 128
                    qo = max(0, d) * 128
                    if j == 0:
                        po_live[h] = psO.tile([128, 512], F32, tag="po", name="po")
                    nc.tensor.matmul(
                        po_live[h][:, qo:512],
                        lhsT=v_sb[j][:, h, :],
                        rhs=pP_live[(g, j, hp)][:, e * 512:e * 512 + Nj],
                        start=(j == 0),
                        stop=(j == 4 * g + 3),
                    )

            def norm(g, h):
                po = po_live.pop(h)
                # reciprocal_approx_fast can't read cross-lane; stage the
                # denominator row at partition 0 (DVE copy handles the shift).
                rden = small.tile([1, 512], F32, tag="rden", name="rden")
                nc.vector.tensor_copy(out=rden, in_=po[D:D + 1, :])
                rec = small.tile([1, 512], F32, tag="rec", name="rec")
                nc.vector.reciprocal_approx_fast(rec, rden)
                bc = small.tile([64, 512], F32, tag="bc", name="bc")
                nc.gpsimd.partition_broadcast(bc, rec)
                gsl = bass.ts(g, 512)
                hb, par = h // 2, h % 2
                if par == 0:
                    nc.vector.tensor_mul(yT128[hb][0:64, gsl], po[0:D, :], bc)
                else:
                    # odd heads land on rows 64-127 of yT128; DVE can't shift
                    # partitions, so stage in a 64-row tile and DMA across.
                    yodd = small.tile([64, 512], BF16, tag="yodd", name="yodd")
                    nc.vector.tensor_mul(yodd, po[0:D, :], bc)
                    nc.sync.dma_start(out=yT128[hb][64:128, gsl], in_=yodd)

            def proj(g, tb):
                tsl = bass.ts(tb, 128)
                osb = pout.tile([128, C], BF16, tag="osb", name="osb")
                for oci in range(2):
                    oc0 = oci * 384
                    mm = psB.tile([128, 384], F32, tag="mm", name="mm")
                    for fb in range(3):
                        nc.tensor.matmul(
                            mm,
                            lhsT=yT128[fb][:, tsl],
                            rhs=wp_sb[:, fb, oc0:oc0 + 384],
                            start=(fb == 0),
                            stop=(fb == 2),
                        )
                    nc.vector.tensor_copy(out=osb[:, oc0:oc0 + 384], in_=mm)
                nc.gpsimd.dma_start(out=out[tsl, :], in_=osb)

            def av_pair_units(g, hp):
                units = []
                J = 4 * g + 4
                for h in (2 * hp, 2 * hp + 1):
                    for j0 in range(0, J, 4):
                        js = list(range(j0, min(j0 + 4, J)))
                        units.append(lambda g=g, h=h, js=js: av_unit(g, h, js))
                    units.append(lambda g=g, h=h: norm(g, h))
                return units

            # ---- lead-in: qkv for t-blocks 0..3 ----
            for tb in range(4):
                for chunk in range(3):
                    qkv_chunk(tb, chunk)

            # one-time padding, emitted after the lead-in so the gpsimd
            # queue serves the rope tables first
            nc.gpsimd.memset(kTp_v[0:64, :, 1:2, :], 0.0)
            nc.gpsimd.memset(kTp_v[64:128, :, 0:1, :], 0.0)
            for i in range(TB):
                nc.gpsimd.memset(v_sb[i][:, :, D:D + 1], 1.0)
                nc.gpsimd.memset(v_sb[i][:, :, D + 1:128], 0.0)

            # ---- slabs, head-pair-major ----
            for g in range(NSLAB):
                stage_x(g + 2)
                stage_sincos(g + 2)
                J = 4 * g + 4
                for hp in range(3):
                    shadow = []
                    # previous pair's attention-V (same slab; pair 2 of the
                    # previous slab when hp == 0)
                    if hp >= 1:
                        shadow += av_pair_units(g, hp - 1)
                    elif g >= 1:
                        shadow += av_pair_units(g - 1, 2)
                    # a third of the next qkv group per stream
                    if g < NSLAB - 1:
                        tbs = range(4 * (g + 1), 4 * (g + 1) + 4)
                        ch = [(tb, c) for tb in tbs for c in range(3)]
                        for tb, c in ch[hp * 4:(hp + 1) * 4]:
                            shadow.append(lambda tb=tb, c=c: qkv_chunk(tb, c))
                    # previous slab's projection in streams 1 and 2
                    if g >= 1 and hp >= 1:
                        for tb in range(4 * (g - 1) + (hp - 1) * 2,
                                        4 * (g - 1) + (hp - 1) * 2 + 2):
                            shadow.append(lambda g=g, tb=tb: proj(g - 1, tb))
                    total = len(shadow)
                    Jeff = max(1, J - 1)
                    done = 0
                    for j in range(J):
                        scores(g, j, hp)
                        want = min(total, math.ceil(total * (j + 1) / Jeff))
                        while done < want:
                            shadow[done]()
                            done += 1

            # ---- tail: last pair's attention-V + last slab projection ----
            for u in av_pair_units(NSLAB - 1, 2):
                u()
            for tb in range(12, 16):
                proj(NSLAB - 1, tb)

    nc.finalize()
    return nc


def _bf16(a):
    return np.ascontiguousarray(np.asarray(a)).astype(ml_dtypes.bfloat16)


# permutation putting rope pairs into contiguous even/odd halves per head
_PERM64 = np.concatenate([np.arange(0, D, 2), np.arange(1, D, 2)])


def _prep_core(c, x, w_qkv, w_proj, sin_rep, cos_rep, consts_m):
    b, hh = c // 2, c % 2
    wq = w_qkv[0 * C + hh * 384: 0 * C + hh * 384 + 384].reshape(HL, D, C)
    wk = w_qkv[1 * C + hh * 384: 1 * C + hh * 384 + 384].reshape(HL, D, C)
    wv = w_qkv[2 * C + hh * 384: 2 * C + hh * 384 + 384]
    wq = wq[:, _PERM64, :].reshape(HL * D, C)
    wk = wk[:, _PERM64, :].reshape(HL * D, C)
    w_local = np.concatenate([wq, wk, wv], 0)       # (1152, 768)
    return {
        "xT": _bf16(x[b].T),
        "wqkvT": _bf16(w_local.T),
        "wpT": _bf16(w_proj[:, hh * 384: hh * 384 + 384].T),
        "sinr": sin_rep,
        "cosr": cos_rep,
        "consts": consts_m,
    }


def kernel(x, w_qkv, w_proj, rope_sin, rope_cos, _trace=False):
    global _CACHED_NC
    x = np.asarray(x, dtype=np.float32)
    w_qkv = np.asarray(w_qkv, dtype=np.float32)
    w_proj = np.asarray(w_proj, dtype=np.float32)
    rope_sin = np.asarray(rope_sin, dtype=np.float32)
    rope_cos = np.asarray(rope_cos, dtype=np.float32)

    # (T, 384): per head block [table(32) | table(32)]
    sin_rep = _bf16(np.tile(np.concatenate([rope_sin, rope_sin], 1), (1, HL)))
    cos_rep = _bf16(np.tile(np.concatenate([rope_cos, rope_cos], 1), (1, HL)))
    tri_m = np.arange(128)[:, None] > np.arange(128)[None, :]
    negi_m = np.eye(128) * -1e9
    ident_m = np.eye(128)
    consts_m = _bf16(np.concatenate([tri_m, negi_m, ident_m], axis=1))

    in_maps = [_prep_core(c, x, w_qkv, w_proj, sin_rep, cos_rep, consts_m)
               for c in range(NCORES)]

    if _CACHED_NC is None:
        _CACHED_NC = build_nc()
    nc = _CACHED_NC

    try:
        res = run_bass_kernel_spmd(nc, in_maps, core_ids=list(range(NCORES)),
                                   trace=_trace)
    except Exception:
        # transient NRT_EXEC_UNIT_UNRECOVERABLE: one retry recovers
        res = run_bass_kernel_spmd(nc, in_maps, core_ids=list(range(NCORES)),
                                   trace=_trace)
    parts = [res.results[c]["out"].astype(np.float32) for c in range(NCORES)]
    out = np.stack([parts[2 * b] + parts[2 * b + 1] for b in range(B)], 0)
    if _trace:
        return out.astype(np.float32), res
    return out.astype(np.float32)
